# revision 27
# baseline (speedup 1.0000x reference)
"""Trainium2 Bass kernel for nn_Attention_75788992906123.

Reference computation (per batch element, B=8 sharded across 8 cores):
  qkv = w_qkv @ x                        (1x1 conv, 192 -> 384 channels)
  qkv = dwconv3x3(qkv, w_dw)             (per-channel 3x3, SAME zero pad)
  q, v = split(qkv); q = q / ||q||_spatial
  attn = softmax(temp * q @ q^T) per head (4 heads x 48 chans)
  out  = attn @ v ; y = w_proj @ out

Kernel strategy per core (batch element):
  - qkv as float32r matmuls (full PE rate, ~1e-4 precision), streamed over
    8 h-strips of 16 rows (+1 row halo each side for the conv).
  - depthwise conv as 9 per-channel taps: q chunks on the PE as accumulating
    diag-weight bf16 matmuls (shifted/restricted 3D APs give exact SAME zero
    padding in PSUM); the v chunk as DVE scalar_tensor_tensor FMAs with an
    ACT-initialized center tap.
  - Gram G = q_raw @ q_raw^T accumulated in PSUM over all spatial chunks
    (PE transposes q strips into [n,c] bf16 tiles first).  Norms come from
    diag(G), so normalized q is never materialized.
  - logits = temp * s_c * s_d * G via two row-scales around a PE transpose
    (G_head is symmetric).  softmax over the 48-wide head block.
  - w_proj folded into M = w_proj_h @ attn_h, final y = M @ v as one bf16
    matmul over the SBUF-resident bf16 v; y ships as bf16 and the host casts
    back to fp32.

  The walrus build here accepts only ONE sync-wait per instruction, so the
  Tile-scheduled program is post-processed by _split_multi_waits.
"""

import sys

if "/opt/trn_rl_repo" not in sys.path:
    sys.path.insert(0, "/opt/trn_rl_repo")

from contextlib import ExitStack

import numpy as np
import ml_dtypes

import concourse.bass as bass
import concourse.mybir as mybir
import concourse.tile as tile
from concourse.bass_utils import run_bass_kernel_spmd
from concourse.masks import make_identity

F32 = mybir.dt.float32
F32R = mybir.dt.float32r
BF16 = mybir.dt.bfloat16
FP8 = mybir.dt.float8e4
DRM = mybir.MatmulPerfMode.DoubleRow
AF = mybir.ActivationFunctionType
ALU = mybir.AluOpType

DIM, H, W = 192, 128, 128
N = H * W          # 16384 spatial
HEADS, HC = 4, 48  # head channels
S = 8              # h-strips per core
SR = H // S        # 16 rows per strip
SCOLS = SR * W     # 2048 cols per strip
NB = 512           # n-block for big matmuls

_wsplit_n = [0]


def _split_multi_waits(nc):
    """walrus here accepts only ONE sync-wait per instruction; hoist extras
    into standalone same-engine NoOps placed immediately before."""
    for _, bbx in nc.bb_map.items():
        insts = bbx.bb.instructions
        out = []
        changed = False
        for inst in insts:
            si = inst.sync_info
            if si is not None and len(si.on_wait) > 1:
                waits = list(si.on_wait)
                for w in waits[:-1]:
                    _wsplit_n[0] += 1
                    nop = mybir.InstNoOp(name=f"I-wsplit{_wsplit_n[0]}")
                    nop.engine = inst.engine
                    nop.sync_info = mybir.SyncInfo(on_wait=[w], on_update=[])
                    out.append(nop)
                si.on_wait = [waits[-1]]
                changed = True
            out.append(inst)
        if changed:
            bbx.bb.instructions = out


def _issue_transpose(nc, qtp, qv, qt_hist, s):
    """q^T for strip s via xbar DMA transpose: [c, n] -> [128 x, 16 j, 192 c]."""
    qt = qtp.tile([128, SR * DIM], BF16, tag="qt", name=f"qt_{s}")
    qt3 = qt[:].rearrange("p (j c) -> p j c", j=SR, c=DIM)
    nc.sync.dma_start_transpose(qt3[:, :, 0:64], qv[0][:, 0:SCOLS])
    nc.sync.dma_start_transpose(qt3[:, :, 64:128], qv[0][:, SCOLS:2 * SCOLS])
    nc.sync.dma_start_transpose(qt3[:, :, 128:192], qv[1][0:64, :])
    qt_hist[s] = qt


def _issue_gram(nc, qt, g_a, g_b, s):
    qt3 = qt[:].rearrange("p (j c) -> p j c", j=SR, c=DIM)
    for j in range(SR):
        bi = SR * s + j
        nc.tensor.matmul(g_a, qt3[:, j, 0:96], qt3[:, j, 0:96],
                         start=(bi == 0), stop=(bi == N // W - 1))
        nc.tensor.matmul(g_b, qt3[:, j, 96:192], qt3[:, j, 96:192],
                         start=(bi == 0), stop=(bi == N // W - 1))


def _strip_geom(s):
    us = max(0, SR * s - 1)
    ue = min(H, SR * s + SR + 1)
    return us, ue - us  # start row, number of u rows


def _qkv_blocks(ucols):
    """split ucols into matmul n-blocks, each >=256 (f32r full rate)."""
    blocks = []
    off = 0
    while ucols - off > 0:
        b = 384 if ucols - off >= 384 else ucols - off
        blocks.append((off, b))
        off += b
    return blocks


def build_nc(conv_cfg=(0, 0)):
    nc = bass.Bass("TRN2", num_devices=8)

    x_d = nc.declare_dram_parameter("x", [DIM, N], F32, isOutput=False)
    wq_d = nc.declare_dram_parameter("wqkvT", [DIM, 2 * DIM], F32, isOutput=False)
    w9_d = nc.declare_dram_parameter("w9", [128, 27], F32, isOutput=False)
    wp_d = nc.declare_dram_parameter("wprojT", [HC, 4 * DIM], BF16, isOutput=False)
    tp_d = nc.declare_dram_parameter("tempv", [96, 2], F32, isOutput=False)
    am_d = nc.declare_dram_parameter("amask", [96, 96], F32, isOutput=False)
    n_pe, n_gps = conv_cfg
    # atom a = s*3 + m.  The q chunks (m=0,1) feed PE transposes, so they go
    # on the PE (no cross-engine wait); the v chunk (m=2) feeds only the
    # GPSIMD convert + DMA spill, so it lives on DVE.  n_pe beyond 16 moves
    # some m=2 strips onto the PE too; n_gps moves trailing m=2 strips to GPSIMD.
    pe_atoms = set()
    if n_pe:
        pe_atoms = {s * 3 + m for s in range(S) for m in (0, 1)}
        for s in range(max(0, n_pe - 16)):
            pe_atoms.add(s * 3 + 2)
        for s in range(max(0, 16 - n_pe)):  # below 16: move m=1 atoms to DVE
            pe_atoms.discard((S - 1 - s) * 3 + 1)
    gps_atoms = {s * 3 + 2 for s in range(S - n_gps, S)} - pe_atoms if n_gps else set()
    dg_d = nc.declare_dram_parameter("diagw", [128, 18 * 128],
                                     BF16, isOutput=False) if n_pe else None
    # fp8 DoubleRow conv weights for m=0: 12 pair tiles [128, 2*64] + 11 diag
    # singles [128, 128] (9 taps + negated t3/t5 for wraparound-col cancel).
    f8_d = nc.declare_dram_parameter("f8w", [128, 25 * 128], FP8, isOutput=False) if n_pe else None
    y_d = nc.declare_dram_parameter("y", [DIM, N], BF16, isOutput=True)

    with tile.TileContext(nc) as tc:
        with ExitStack() as ctx:
            wp = ctx.enter_context(tc.tile_pool(name="wp", bufs=1))
            xp = ctx.enter_context(tc.tile_pool(name="xp", bufs=2))
            up = ctx.enter_context(tc.tile_pool(name="up", bufs=2))
            qvp = ctx.enter_context(tc.tile_pool(name="qvp", bufs=2))
            vsp = ctx.enter_context(tc.tile_pool(name="vsp", bufs=1))
            qtp = ctx.enter_context(tc.tile_pool(name="qtp", bufs=2))
            yp = ctx.enter_context(tc.tile_pool(name="yp", bufs=2))
            pmm = ctx.enter_context(tc.tile_pool(name="pmm", bufs=3, space="PSUM"))
            pcv = ctx.enter_context(tc.tile_pool(name="pcv", bufs=2, space="PSUM"))
            pg = ctx.enter_context(tc.tile_pool(name="pg", bufs=1, space="PSUM"))

            # ---- strip-0 x prefetch first: nothing computes until it lands ----
            x_pre = {}
            us0, un0 = _strip_geom(0)
            x0p = xp.tile([128, 2304], F32R, tag="x0", name="x0_pre")
            x1p = xp.tile([64, 2304], F32R, tag="x1", name="x1_pre")
            uc0 = un0 * W
            cuts0 = [(uc0 * i // 4 // W) * W for i in range(4)] + [uc0]
            wq0 = wp.tile([128, 2 * DIM], F32R, tag="wq0")
            wq1 = wp.tile([64, 2 * DIM], F32R, tag="wq1")
            for ci in range(4):
                c0, c1 = cuts0[ci], cuts0[ci + 1]
                nc.sync.dma_start(x0p[:, c0:c1], x_d[0:128, c0:c1].bitcast(F32R))
                if ci == 0:
                    nc.sync.dma_start(wq0[:], wq_d[0:128, :].bitcast(F32R))
                    nc.sync.dma_start(wq1[:], wq_d[128:192, :].bitcast(F32R))
                nc.sync.dma_start(x1p[:, c0:c1], x_d[128:192, c0:c1].bitcast(F32R))
            x_pre[0] = (x0p, x1p)
            w9t = wp.tile([128, 27], F32, tag="w9t")
            nc.sync.dma_start(w9t[:], w9_d[:])
            w9 = [w9t[:, 9 * c:9 * (c + 1)] for c in range(3)]
            wptt = wp.tile([HC, 4 * DIM], BF16, tag="wptt")
            nc.sync.dma_start(wptt[:], wp_d[:])
            wpt = [wptt[:, DIM * h:DIM * (h + 1)] for h in range(HEADS)]
            tvt = wp.tile([96, 2], F32, tag="tvt")
            nc.sync.dma_start(tvt[:], tp_d[:])
            tempv = {"a": tvt[:, 0:1], "b": tvt[:, 1:2]}
            amask = wp.tile([96, 96], F32, tag="amask")
            nc.sync.dma_start(amask[:], am_d[:])
            ident = wp.tile([128, 128], F32, tag="ident")
            make_identity(nc, ident[:])
            dgt = {}
            dr8t = {}
            dg8t = {}
            if n_pe:
                dgall = wp.tile([128, 18 * 128], BF16, tag="dgall")
                nc.sync.dma_start(dgall[:], dg_d[:])
                for c in (1, 2):
                    for t in range(9):
                        i = (c - 1) * 9 + t
                        dgt[(c, t)] = dgall[:, 128 * i:128 * (i + 1)]
                f8w = wp.tile([128, 25 * 128], FP8, tag="f8w")
                nc.sync.dma_start(f8w[:], f8_d[:])
                for i in range(14):
                    dr8t[i] = f8w[:, 128 * i:128 * (i + 1)].rearrange("p (k m) -> p k m", k=2)
                for i in range(11):
                    dg8t[i] = f8w[:, 128 * (14 + i):128 * (15 + i)]

            # bf16 v resident in SBUF (rows 64:128 of vb_a carry v chans 0:64)
            vb_a = vsp.tile([128, N], BF16, tag="vb_a", bufs=1)
            vb_b = vsp.tile([128, N], BF16, tag="vb_b", bufs=1)
            # Gram psums share one bank: [96, 384] = cols 0:192 chunk A, 192:384 B
            g_all = pg.tile([96, DIM], F32, tag="g_all")
            g_a = g_all[:, 0:96]
            g_b = g_all[:, 96:192]

            # center tap first: it initializes the accumulator over the full
            # strip; the restricted edge taps then accumulate into subsets.
            TAPS = [(0, 0)] + [(dy, dx) for dy in (-1, 0, 1) for dx in (-1, 0, 1)
                               if (dy, dx) != (0, 0)]

            def _load_x(s):
                us, un = _strip_geom(s)
                ucols = un * W
                x0 = xp.tile([128, 2304], F32R, tag="x0", name=f"x0_{s}")
                x1 = xp.tile([64, 2304], F32R, tag="x1", name=f"x1_{s}")
                cuts = [(ucols * i // 2 // W) * W for i in range(2)] + [ucols]
                for ci in range(2):
                    c0, c1 = cuts[ci], cuts[ci + 1]
                    nc.sync.dma_start(x0[:, c0:c1], x_d[0:128, us * W + c0:us * W + c1].bitcast(F32R))
                    nc.sync.dma_start(x1[:, c0:c1], x_d[128:192, us * W + c0:us * W + c1].bitcast(F32R))
                x_pre[s] = (x0, x1)

            qv_hist = {}
            qt_hist = {}
            _emit_gram_late = [None]
            for s in range(S):
                us, un = _strip_geom(s)
                ucols = un * W
                # ---- prefetch next strip's x; this strip's was loaded earlier ----
                if s + 1 < S:
                    _load_x(s + 1)
                x0, x1 = x_pre[s]
                # ---- qkv matmuls -> u strip (fp32) ----
                # m=0 (pure-q chans) lands as fp8 with a 1-elem leading pad so
                # full-width shifted conv APs never go below offset 0.
                u = [up.tile([128, 2306], FP8, tag="u0", name=f"u_{s}_0")] + \
                    [up.tile([128, 2304], BF16, tag=f"u{m}", name=f"u_{s}_{m}") for m in (1, 2)]
                nc.vector.memset(u[0][:, 0:1], 0.0)
                nc.vector.memset(u[0][:, 1 + ucols:2306], 0.0)
                for m in range(3):
                    doff = 1 if m == 0 else 0
                    for (boff, bn) in _qkv_blocks(ucols):
                        ps = pmm.tile([128, NB], F32, tag="mm")
                        nc.tensor.matmul(ps[:, 0:bn], wq0[:, 128 * m:128 * (m + 1)],
                                         x0[:, boff:boff + bn], start=True, stop=False)
                        nc.tensor.matmul(ps[:, 0:bn], wq1[:, 128 * m:128 * (m + 1)],
                                         x1[:, boff:boff + bn], start=False, stop=True)
                        if m == 2 and (s * 3 + 2) not in pe_atoms:
                            nc.vector.tensor_copy(u[m][:, boff:boff + bn], ps[:, 0:bn])
                        else:
                            nc.scalar.activation(u[m][:, doff + boff:doff + boff + bn], ps[:, 0:bn], AF.Copy)
                # ---- depthwise conv: qv[c, r, w] = sum_t w9[c,t]*u[c, r+dy, w+dx] ----
                qv = [qvp.tile([64, 2 * SCOLS], BF16, tag="qv0", name=f"qv_{s}_0"),
                      qvp.tile([128, SCOLS], BF16, tag="qv1", name=f"qv_{s}_1"),
                      None]
                qv_hist[s] = qv
                for m in range(3):
                    uv = u[m][:, 0:ucols].rearrange("p (h w) -> p h w", h=un, w=W) if m else None
                    ovt = (vb_b[:, s * SCOLS:(s + 1) * SCOLS] if m == 2 else
                           (qv[m][:] if m else None))
                    ov = ovt.rearrange("p (h w) -> p h w", h=SR, w=W) if m else None
                    atom = s * 3 + m
                    if m == 0 and atom in pe_atoms:
                        # fp8 DoubleRow conv, wide layout: DR outputs must sit at
                        # partition base 0, so both 64-chan groups write psum
                        # [64, 1024] (g0 cols 0:512, g1 cols 512:1024).  dy-tap
                        # pairs + (0,-1)/(0,+1) + (center,zero) run full-width;
                        # wraparound cols cancelled by negated-weight matmuls.
                        u0f = u[0][:]
                        APc = type(u0f)
                        ppu = list(list(u0f.ap)[0])

                        def sap(off, dims):
                            return APc(u0f.tensor, off, [ppu] + [list(d) for d in dims])

                        for boff in range(0, SCOLS, NB):
                            r0 = boff // W
                            bh = NB // W
                            ps = pcv.tile([64, 2 * NB], F32, tag="cv", name=f"cps_{s}_0_{boff}")
                            psf = ps[:]
                            ppp = list(list(psf.ap)[0])

                            def spp(off, dims):
                                return APc(psf.tensor, off, [ppp] + [list(d) for d in dims])

                            a0 = SR * s + r0 - us
                            # center (paired with zero k-tile, stride-0 rhs)
                            icen = sap(1 + a0 * W, [[0, 2], [1, NB]])
                            seq = [(ps[:, g * NB:(g + 1) * NB], dr8t[8 + g], icen, DRM)
                                   for g in range(2)]
                            for pi, dx in ((0, -1), (1, 0), (2, 1)):
                                loA = max(r0, max(0, -SR * s + 1))
                                hiA = min(r0 + bh, min(SR, H - SR * s + 1))
                                loB, hiB = r0, min(r0 + bh, min(SR, H - SR * s - 1))
                                lo, hi = max(loA, loB), min(hiA, hiB)
                                ws_o, ws_i, wc = (0, 0, W) if dx == 0 else ((0, 1, W - 1) if dx == 1 else (1, 0, W - 1))
                                if hi > lo:
                                    h = hi - lo
                                    aA = SR * s + lo - 1 - us
                                    i4 = sap(1 + aA * W + dx, [[2 * W, 2], [1, h * W]])
                                    for g in range(2):
                                        seq.append((ps[:, g * NB + (lo - r0) * W:g * NB + (hi - r0) * W],
                                                    dr8t[2 * pi + g], i4, DRM))
                                    if dx == -1:
                                        i3 = sap(aA * W, [[2 * W, 2], [W, h]])
                                        for g in range(2):
                                            seq.append((spp(g * NB + (lo - r0) * W, [[W, h]]),
                                                        dr8t[10 + g], i3, DRM))
                                    elif dx == 1:
                                        i3 = sap(1 + (aA + 1) * W, [[2 * W, 2], [W, h]])
                                        for g in range(2):
                                            seq.append((spp(g * NB + (lo - r0) * W + W - 1, [[W, h]]),
                                                        dr8t[12 + g], i3, DRM))
                                # leftover rows where one dy-tap is clamped
                                for dy_, lo_, hi_ in ((-1, loA, hiA), (1, loB, hiB)):
                                    t_ = (dy_ + 1) * 3 + (dx + 1)
                                    for y0, y1 in ((lo_, lo), (hi, hi_)):
                                        for y in range(y0, y1):
                                            a_ = SR * s + y + dy_ - us
                                            for g in range(2):
                                                seq.append((ps[:, g * NB + (y - r0) * W + ws_o:g * NB + (y - r0) * W + ws_o + wc],
                                                            dg8t[t_][:, g * 64:(g + 1) * 64],
                                                            u[0][:, 1 + a_ * W + ws_i:1 + a_ * W + ws_i + wc], None))
                            # (0,-1)+(0,+1) full-width pair, k-stride 2
                            i4 = sap(a0 * W, [[2, 2], [1, bh * W]])
                            for g in range(2):
                                seq.append((ps[:, g * NB:(g + 1) * NB], dr8t[6 + g], i4, DRM))
                            i3l = sap(a0 * W, [[W, bh]])
                            i3r = sap(1 + (a0 + 1) * W, [[W, bh]])
                            for g in range(2):
                                seq.append((spp(g * NB, [[W, bh]]), dg8t[9][:, g * 64:(g + 1) * 64], i3l, None))
                                seq.append((spp(g * NB + W - 1, [[W, bh]]), dg8t[10][:, g * 64:(g + 1) * 64], i3r, None))
                            for ii, (o_, w_, i_, pm_) in enumerate(seq):
                                nc.tensor.matmul(o_, w_, i_, start=(ii < 2), stop=(ii >= len(seq) - 2),
                                                 perf_mode=pm_, skip_group_check=True)
                            ps3 = ps[:].rearrange("p (g c) -> p g c", g=2)
                            qv03 = qv[0][:].rearrange("p (g c) -> p g c", g=2)
                            nc.scalar.activation(qv03[:, :, boff:boff + NB], ps3, AF.Copy)
                        continue
                    if atom in pe_atoms:
                        # PE: 9 accumulating f32r diag matmuls per 512-block,
                        # shifted/restricted 3D APs give exact zero padding.
                        for boff in range(0, SCOLS, NB):
                            r0 = boff // W
                            ps = pcv.tile([128, NB], F32, tag="cv", name=f"cps_{s}_{m}_{boff}")
                            pv = ps[:].rearrange("p (h w) -> p h w", h=NB // W, w=W)
                            for ti, (dy, dx) in enumerate(TAPS):
                                t = (dy + 1) * 3 + (dx + 1)
                                lo = max(r0, max(0, -SR * s - dy))
                                hi = min(r0 + NB // W, min(SR, H - SR * s - dy))
                                if hi <= lo:
                                    continue
                                hc = hi - lo
                                if dx == 0:
                                    ws_o, ws_i, wc = 0, 0, W
                                elif dx == 1:
                                    ws_o, ws_i, wc = 0, 1, W - 1
                                else:
                                    ws_o, ws_i, wc = 1, 0, W - 1
                                a = SR * s + lo + dy - us
                                o_ap = pv[:, lo - r0:hi - r0, ws_o:ws_o + wc]
                                i_ap = uv[:, a:a + hc, ws_i:ws_i + wc]
                                if dy == 0 and dx == 0:
                                    # flat full-block stream (covers whole psum,
                                    # sets has_written everywhere); center tap
                                    # has lo == r0, so u row a aligns with r0.
                                    o_use = ps[:]
                                    i_use = u[m][:, a * W:a * W + NB]
                                else:
                                    o_use, i_use = o_ap, i_ap
                                nc.tensor.matmul(o_use, dgt[(m, t)], i_use,
                                                 start=(ti == 0), stop=(ti == len(TAPS) - 1))
                            dst_bb = ovt[:, boff:boff + NB] if m == 2 else qv[m][:, boff:boff + NB]
                            nc.scalar.activation(dst_bb, ps[:], AF.Copy)
                        continue
                    stt = nc.gpsimd.scalar_tensor_tensor if atom in gps_atoms else nc.vector.scalar_tensor_tensor
                    # center tap initializes the accumulator on ACT (single-src
                    # scale runs 2x there and frees the DVE)
                    a0 = SR * s - us
                    nc.scalar.activation(ovt, u[m][:, a0 * W:a0 * W + SCOLS], AF.Copy,
                                         scale=w9[m][:, 4:5])
                    for (dy, dx) in TAPS[1:]:
                        t = (dy + 1) * 3 + (dx + 1)
                        r_lo = max(0, -SR * s - dy)
                        r_hi = min(SR, H - SR * s - dy)
                        hc = r_hi - r_lo
                        if dx == 0:
                            ws_o, ws_i, wc = 0, 0, W
                        elif dx == 1:
                            ws_o, ws_i, wc = 0, 1, W - 1
                        else:
                            ws_o, ws_i, wc = 1, 0, W - 1
                        a = SR * s + r_lo + dy - us
                        o_ap = ov[:, r_lo:r_hi, ws_o:ws_o + wc]
                        i_ap = uv[:, a:a + hc, ws_i:ws_i + wc]
                        sc = w9[m][:, t:t + 1]
                        stt(o_ap, i_ap, sc, o_ap, op0=ALU.mult, op1=ALU.add)
                # conv tap order: center (init) must run first; Tile's RAW/WAW
                # tracking keeps the remaining taps ordered on the accumulator.
                # ---- v -> resident bf16 ----
                nc.gpsimd.tensor_copy(vb_a[64:128, s * SCOLS:(s + 1) * SCOLS], qv[1][64:128, :])
                if _emit_gram_late[0] is not None:
                    _issue_gram(nc, qt_hist[_emit_gram_late[0]], g_a, g_b, _emit_gram_late[0])
                    _emit_gram_late[0] = None
                # ---- lagged q^T (xbar DMA transpose); gram emitted at the
                # end of the strip so the PE reaches it a conv-duration later.
                if s >= 1:
                    _issue_transpose(nc, qtp, qv_hist[s - 1], qt_hist, s - 1)
                    _emit_gram_late[0] = s - 1

            # flush the lagged transpose/gram pipeline
            _issue_transpose(nc, qtp, qv_hist[S - 1], qt_hist, S - 1)
            _issue_gram(nc, qt_hist[S - 1], g_a, g_b, S - 1)

            # ================= attention head math =================
            gs_all = wp.tile([96, 2 * 96], F32, tag="gs_all")
            nc.scalar.activation(gs_all[:], g_all[:], AF.Copy)
            gs = {"a": gs_all[:, 0:96], "b": gs_all[:, 96:192]}
            st = {}
            sonly = {}
            for key in ("a", "b"):
                # diag -> norms: d_sq[r] = sum_c G[r, c] * I[r, c]
                dtmp = wp.tile([96, 96], F32, tag=f"dtmp_{key}")
                nc.vector.tensor_mul(dtmp[:], gs[key], ident[0:96, 0:96])
                dsq = wp.tile([96, 1], F32, tag=f"dsq_{key}")
                nc.vector.tensor_reduce(dsq[:], dtmp[:], axis=mybir.AxisListType.X, op=ALU.add)
                dn = wp.tile([96, 1], F32, tag=f"dn_{key}")
                nc.scalar.sqrt(dn[:], dsq[:])
                sv = wp.tile([96, 1], F32, tag=f"sv_{key}")
                nc.vector.reciprocal(sv[:], dn[:])
                sonly[key] = sv
                stv = wp.tile([96, 1], F32, tag=f"stv_{key}")
                nc.vector.tensor_mul(stv[:], sv[:], tempv[key])
                st[key] = stv

            at_bf = {}
            for key in ("a", "b"):
                # mask first: -1e30 on cross-head blocks survives the
                # positive scales/transposes (exp -> 0 either way) and runs
                # concurrently with the norm chain instead of after it.
                gm = wp.tile([96, 96], F32, tag=f"gm_{key}")
                nc.vector.tensor_add(gm[:], gs[key], amask[:])
                # Z = diag(s*temp) @ (G_block + mask)   [96, 96]
                z = wp.tile([96, 96], F32, tag=f"z_{key}")
                nc.scalar.activation(z[:], gm[:], AF.Copy, scale=st[key][:])
                pzt = pcv.tile([128, 512], F32, tag="cv", name="pzt")
                nc.tensor.transpose(pzt[0:96, 0:96], z[:], ident[0:96, 0:96])
                zt = wp.tile([96, 96], F32, tag=f"zt_{key}")
                nc.scalar.activation(zt[:], pzt[0:96, 0:96], AF.Copy, scale=sonly[key][:])
                pl = pcv.tile([128, 512], F32, tag="cv", name="pl")
                nc.tensor.transpose(pl[0:96, 0:96], zt[:], ident[0:96, 0:96])
                e = wp.tile([96, 96], F32, tag=f"e_{key}")
                nc.scalar.activation(e[:], pl[0:96, 0:96], AF.Exp)
                rs = wp.tile([96, 1], F32, tag=f"rs_{key}")
                nc.vector.tensor_reduce(rs[:], e[:], axis=mybir.AxisListType.X, op=ALU.add)
                rc = wp.tile([96, 1], F32, tag=f"rc_{key}")
                nc.vector.reciprocal(rc[:], rs[:])
                ab = wp.tile([96, 96], BF16, tag=f"ab_{key}")
                nc.scalar.activation(ab[:], e[:], AF.Copy, scale=rc[:])
                at_bf[key] = ab

            # move odd-head attn blocks to partition 0 (PE needs 32-aligned base)
            scr1 = wp.tile([HC, HC], BF16, tag="scr1")
            scr3 = wp.tile([HC, HC], BF16, tag="scr3")
            nc.sync.dma_start(scr1[:], at_bf["a"][48:96, 48:96])
            nc.sync.dma_start(scr3[:], at_bf["b"][48:96, 48:96])

            # M_h^T = attn_h^T-contracted with w_projT rows: [48d, 192o]
            mh_s = []
            for h in range(HEADS):
                if h == 0:
                    lhsT = at_bf["a"][0:48, 0:48]
                elif h == 1:
                    lhsT = scr1[:]
                elif h == 2:
                    lhsT = at_bf["b"][0:48, 0:48]
                else:
                    lhsT = scr3[:]
                pm = pcv.tile([128, 512], F32, tag="cv", name=f"pm{h}")
                nc.tensor.matmul(pm[0:HC, 0:DIM], lhsT, wpt[h], start=True, stop=True)
                ms = wp.tile([HC, DIM], BF16, tag=f"mh{h}")
                nc.scalar.activation(ms[:], pm[0:HC, 0:DIM], AF.Copy)
                mh_s.append(ms)

            # stack into final lhsT k-chunks: Ma rows 64:128 = d 0:64 (base
            # partition must match vb_a), Mb = d 64:192
            m_a = wp.tile([128, DIM], BF16, tag="m_a")
            m_b = wp.tile([128, DIM], BF16, tag="m_b")
            nc.sync.dma_start(m_a[64:112, :], mh_s[0][:])
            nc.gpsimd.dma_start(m_a[112:128, :], mh_s[1][0:16, :])
            nc.sync.dma_start(m_b[0:32, :], mh_s[1][16:48, :])
            nc.gpsimd.dma_start(m_b[32:80, :], mh_s[2][:])
            nc.sync.dma_start(m_b[80:128, :], mh_s[3][:])

            # ================= final y = M @ v =================
            YB = 4 * NB  # 2048-col y chunks -> few large output DMAs
            for goff in range(0, N, YB):
                yts = [yp.tile([128, YB], BF16, tag=f"yt{mi}", name=f"yt{mi}_{goff}")
                       for mi in range(2)]
                for sub in range(4):
                    boff = goff + sub * NB
                    for mi, (mp0, mn) in enumerate(((0, 128), (128, 64))):
                        ps = pmm.tile([128, NB], F32, tag="mm")
                        nc.tensor.matmul(ps[0:mn, :], m_a[64:128, mp0:mp0 + mn],
                                         vb_a[64:128, boff:boff + NB], start=True, stop=False)
                        nc.tensor.matmul(ps[0:mn, :], m_b[:, mp0:mp0 + mn],
                                         vb_b[:, boff:boff + NB], start=False, stop=True)
                        dst = yts[mi][0:mn, sub * NB:(sub + 1) * NB]
                        if mi == 0:
                            nc.scalar.activation(dst, ps[0:mn, :], AF.Copy)
                        else:
                            nc.vector.tensor_copy(dst, ps[0:mn, :])
                for mi, (mp0, mn) in enumerate(((0, 128), (128, 64))):
                    nc.sync.dma_start(y_d[mp0:mp0 + mn, goff:goff + YB], yts[mi][0:mn, :])

    _split_multi_waits(nc)
    return nc


_CACHED = {}


def _get_nc(conv_cfg=(0, 0)):
    key = tuple(conv_cfg)
    if key not in _CACHED:
        _CACHED[key] = build_nc(key)
    return _CACHED[key]


def _host_prep(x, w_qkv, w_dw, w_proj, temperature):
    x = np.ascontiguousarray(np.asarray(x, dtype=np.float32))
    w_qkv = np.asarray(w_qkv, dtype=np.float32)
    w_dw = np.asarray(w_dw, dtype=np.float32)
    w_proj = np.asarray(w_proj, dtype=np.float32)
    temperature = np.asarray(temperature, dtype=np.float32)
    B = x.shape[0]
    xs = x.reshape(B, DIM, N)
    wqkvT = np.ascontiguousarray(w_qkv.T)                      # [192, 384]
    w9 = np.ascontiguousarray(w_dw.reshape(2 * DIM, 9))        # [384, 9]
    w9all = np.zeros((128, 27), np.float32)
    for c in range(3):
        w9all[:, 9 * c:9 * (c + 1)] = w9[128 * c:128 * (c + 1)]
    wprojT = np.ascontiguousarray(w_proj.T.astype(ml_dtypes.bfloat16))  # [192,192]
    wp2 = np.zeros((HC, 4 * DIM), ml_dtypes.bfloat16)
    for h in range(HEADS):
        wp2[:, DIM * h:DIM * (h + 1)] = wprojT[HC * h:HC * (h + 1)]
    tempv = np.repeat(temperature.reshape(HEADS), HC).astype(np.float32)
    tv2 = np.stack([tempv[0:96], tempv[96:192]], axis=1).astype(np.float32)
    return xs, wqkvT, w9, w9all, wp2, tv2


def kernel(x, w_qkv, w_dw, w_proj, temperature, _trace=False, _conv=(16, 0)):
    xs, wqkvT, w9, w9all, wp2, tv2 = _host_prep(x, w_qkv, w_dw, w_proj, temperature)
    B = xs.shape[0]
    nc = _get_nc(_conv)
    in_maps = []
    amask = np.full((96, 96), -1e30, np.float32)
    amask[0:48, 0:48] = 0.0
    amask[48:96, 48:96] = 0.0
    extra = {}
    if _conv[0]:
        dw = np.zeros((18, 128, 128), np.float32)
        for c in (1, 2):
            for t in range(9):
                np.fill_diagonal(dw[(c - 1) * 9 + t], w9[128 * c:128 * (c + 1), t])
        extra["diagw"] = dw.transpose(1, 0, 2).reshape(128, 18 * 128).astype(ml_dtypes.bfloat16)
        # fp8 DoubleRow weights for m=0 (q chans 0:128)
        w0 = w9[0:128].astype(ml_dtypes.float8_e4m3fn).astype(np.float32)
        dg8 = np.zeros((11, 128, 128), np.float32)
        for t in range(9):
            np.fill_diagonal(dg8[t], w0[:, t])
        np.fill_diagonal(dg8[9], -w0[:, 3])
        np.fill_diagonal(dg8[10], -w0[:, 5])
        pairs = [(0, 6), (1, 7), (2, 8), (3, 5), (4, None)]
        dr8 = np.zeros((14, 128, 2, 64), np.float32)
        for pi, (tA, tB) in enumerate(pairs):
            for g in range(2):
                for mc in range(64):
                    p = g * 64 + mc
                    dr8[2 * pi + g, p, 0, mc] = w0[p, tA]
                    if tB is not None:
                        dr8[2 * pi + g, p, 1, mc] = w0[p, tB]
        dr8[10:12] = -dr8[0:2]   # negated dx=-1 pair
        dr8[12:14] = -dr8[4:6]   # negated dx=+1 pair
        f8all = np.concatenate([dr8.reshape(14, 128, 128), dg8], axis=0)
        extra["f8w"] = f8all.transpose(1, 0, 2).reshape(128, 25 * 128).astype(ml_dtypes.float8_e4m3fn)
    for b in range(B):
        m = {"x": xs[b], "wqkvT": wqkvT, "w9": w9all, "wprojT": wp2, "tempv": tv2,
             "amask": amask, **extra}
        in_maps.append(m)
    res = run_bass_kernel_spmd(nc, in_maps, list(range(8)), trace=_trace)
    y = np.stack([np.asarray(res.results[b]["y"]).astype(np.float32) for b in range(B)])
    out = y.reshape(B, DIM, H, W)
    if _trace:
        return out, res
    return out



# revision 30
# speedup vs baseline: 1.0096x; 1.0096x over previous
"""Trainium2 Bass kernel for nn_Attention_75788992906123.

Reference computation (per batch element, B=8 sharded across 8 cores):
  qkv = w_qkv @ x                        (1x1 conv, 192 -> 384 channels)
  qkv = dwconv3x3(qkv, w_dw)             (per-channel 3x3, SAME zero pad)
  q, v = split(qkv); q = q / ||q||_spatial
  attn = softmax(temp * q @ q^T) per head (4 heads x 48 chans)
  out  = attn @ v ; y = w_proj @ out

Kernel strategy per core (batch element):
  - qkv as float32r matmuls (full PE rate, ~1e-4 precision), streamed over
    8 h-strips of 16 rows (+1 row halo each side for the conv).
  - depthwise conv as 9 per-channel taps: q chunks on the PE as accumulating
    diag-weight bf16 matmuls (shifted/restricted 3D APs give exact SAME zero
    padding in PSUM); the v chunk as DVE scalar_tensor_tensor FMAs with an
    ACT-initialized center tap.
  - Gram G = q_raw @ q_raw^T accumulated in PSUM over all spatial chunks
    (PE transposes q strips into [n,c] bf16 tiles first).  Norms come from
    diag(G), so normalized q is never materialized.
  - logits = temp * s_c * s_d * G via two row-scales around a PE transpose
    (G_head is symmetric).  softmax over the 48-wide head block.
  - w_proj folded into M = w_proj_h @ attn_h, final y = M @ v as one bf16
    matmul over the SBUF-resident bf16 v; y ships as bf16 and the host casts
    back to fp32.

  The walrus build here accepts only ONE sync-wait per instruction, so the
  Tile-scheduled program is post-processed by _split_multi_waits.
"""

import sys

if "/opt/trn_rl_repo" not in sys.path:
    sys.path.insert(0, "/opt/trn_rl_repo")

from contextlib import ExitStack

import numpy as np
import ml_dtypes

import concourse.bass as bass
import concourse.mybir as mybir
import concourse.tile as tile
from concourse.bass_utils import run_bass_kernel_spmd
from concourse.masks import make_identity

F32 = mybir.dt.float32
F32R = mybir.dt.float32r
BF16 = mybir.dt.bfloat16
FP8 = mybir.dt.float8e4
DRM = mybir.MatmulPerfMode.DoubleRow
AF = mybir.ActivationFunctionType
ALU = mybir.AluOpType

DIM, H, W = 192, 128, 128
N = H * W          # 16384 spatial
HEADS, HC = 4, 48  # head channels
S = 8              # h-strips per core
SR = H // S        # 16 rows per strip
SCOLS = SR * W     # 2048 cols per strip
NB = 512           # n-block for big matmuls

_wsplit_n = [0]


def _split_multi_waits(nc):
    """walrus here accepts only ONE sync-wait per instruction; hoist extras
    into standalone same-engine NoOps placed immediately before."""
    for _, bbx in nc.bb_map.items():
        insts = bbx.bb.instructions
        out = []
        changed = False
        for inst in insts:
            si = inst.sync_info
            if si is not None and len(si.on_wait) > 1:
                waits = list(si.on_wait)
                for w in waits[:-1]:
                    _wsplit_n[0] += 1
                    nop = mybir.InstNoOp(name=f"I-wsplit{_wsplit_n[0]}")
                    nop.engine = inst.engine
                    nop.sync_info = mybir.SyncInfo(on_wait=[w], on_update=[])
                    out.append(nop)
                si.on_wait = [waits[-1]]
                changed = True
            out.append(inst)
        if changed:
            bbx.bb.instructions = out


def _issue_transpose(nc, qtp, qv, qt_hist, s):
    """q^T for strip s via xbar DMA transpose: [c, n] -> [128 x, 16 j, 192 c]."""
    qt = qtp.tile([128, SR * DIM], BF16, tag="qt", name=f"qt_{s}")
    qt3 = qt[:].rearrange("p (j c) -> p j c", j=SR, c=DIM)
    nc.sync.dma_start_transpose(qt3[:, :, 0:64], qv[0][:, 0:SCOLS])
    nc.sync.dma_start_transpose(qt3[:, :, 64:128], qv[0][:, SCOLS:2 * SCOLS])
    nc.sync.dma_start_transpose(qt3[:, :, 128:192], qv[1][0:64, :])
    qt_hist[s] = qt


def _issue_gram(nc, qt, g_a, g_b, s):
    qt3 = qt[:].rearrange("p (j c) -> p j c", j=SR, c=DIM)
    for j in range(SR):
        bi = SR * s + j
        nc.tensor.matmul(g_a, qt3[:, j, 0:96], qt3[:, j, 0:96],
                         start=(bi == 0), stop=(bi == N // W - 1))
        nc.tensor.matmul(g_b, qt3[:, j, 96:192], qt3[:, j, 96:192],
                         start=(bi == 0), stop=(bi == N // W - 1))


def _strip_geom(s):
    us = max(0, SR * s - 1)
    ue = min(H, SR * s + SR + 1)
    return us, ue - us  # start row, number of u rows


def _qkv_blocks(ucols):
    """split ucols into matmul n-blocks, each >=256 (f32r full rate)."""
    blocks = []
    off = 0
    while ucols - off > 0:
        b = 384 if ucols - off >= 384 else ucols - off
        blocks.append((off, b))
        off += b
    return blocks


def build_nc(conv_cfg=(0, 0)):
    nc = bass.Bass("TRN2", num_devices=8)

    x_d = nc.declare_dram_parameter("x", [DIM, N], F32, isOutput=False)
    wq_d = nc.declare_dram_parameter("wqkvT", [DIM, 2 * DIM], F32, isOutput=False)
    w9_d = nc.declare_dram_parameter("w9", [128, 27], F32, isOutput=False)
    wp_d = nc.declare_dram_parameter("wprojT", [HC, 4 * DIM], BF16, isOutput=False)
    tp_d = nc.declare_dram_parameter("tempv", [96, 2], F32, isOutput=False)
    am_d = nc.declare_dram_parameter("amask", [96, 96], F32, isOutput=False)
    n_pe, n_gps = conv_cfg
    # atom a = s*3 + m.  The q chunks (m=0,1) feed PE transposes, so they go
    # on the PE (no cross-engine wait); the v chunk (m=2) feeds only the
    # GPSIMD convert + DMA spill, so it lives on DVE.  n_pe beyond 16 moves
    # some m=2 strips onto the PE too; n_gps moves trailing m=2 strips to GPSIMD.
    pe_atoms = set()
    if n_pe:
        pe_atoms = {s * 3 + m for s in range(S) for m in (0, 1)}
        for s in range(max(0, n_pe - 16)):
            pe_atoms.add(s * 3 + 2)
        for s in range(max(0, 16 - n_pe)):  # below 16: move m=1 atoms to DVE
            pe_atoms.discard((S - 1 - s) * 3 + 1)
    gps_atoms = {s * 3 + 2 for s in range(S - n_gps, S)} - pe_atoms if n_gps else set()
    dg_d = nc.declare_dram_parameter("diagw", [128, 18 * 128],
                                     BF16, isOutput=False) if n_pe else None
    # fp8 DoubleRow conv weights for m=0: 12 pair tiles [128, 2*64] + 11 diag
    # singles [128, 128] (9 taps + negated t3/t5 for wraparound-col cancel).
    f8_d = nc.declare_dram_parameter("f8w", [128, 25 * 128], FP8, isOutput=False) if n_pe else None
    y_d = nc.declare_dram_parameter("y", [DIM, N], BF16, isOutput=True)

    with tile.TileContext(nc) as tc:
        with ExitStack() as ctx:
            wp = ctx.enter_context(tc.tile_pool(name="wp", bufs=1))
            xp = ctx.enter_context(tc.tile_pool(name="xp", bufs=2))
            up = ctx.enter_context(tc.tile_pool(name="up", bufs=2))
            qvp = ctx.enter_context(tc.tile_pool(name="qvp", bufs=2))
            vsp = ctx.enter_context(tc.tile_pool(name="vsp", bufs=1))
            qtp = ctx.enter_context(tc.tile_pool(name="qtp", bufs=3))
            yp = ctx.enter_context(tc.tile_pool(name="yp", bufs=2))
            pmm = ctx.enter_context(tc.tile_pool(name="pmm", bufs=3, space="PSUM"))
            pcv = ctx.enter_context(tc.tile_pool(name="pcv", bufs=2, space="PSUM"))
            pg = ctx.enter_context(tc.tile_pool(name="pg", bufs=1, space="PSUM"))

            # ---- strip-0 x prefetch first: nothing computes until it lands ----
            x_pre = {}
            us0, un0 = _strip_geom(0)
            x0p = xp.tile([128, 2304], F32R, tag="x0", name="x0_pre")
            x1p = xp.tile([64, 2304], F32R, tag="x1", name="x1_pre")
            uc0 = un0 * W
            cuts0 = [(uc0 * i // 4 // W) * W for i in range(4)] + [uc0]
            wq0 = wp.tile([128, 2 * DIM], F32R, tag="wq0")
            wq1 = wp.tile([64, 2 * DIM], F32R, tag="wq1")
            for ci in range(4):
                c0, c1 = cuts0[ci], cuts0[ci + 1]
                nc.sync.dma_start(x0p[:, c0:c1], x_d[0:128, c0:c1].bitcast(F32R))
                if ci == 0:
                    nc.sync.dma_start(wq0[:], wq_d[0:128, :].bitcast(F32R))
                    nc.sync.dma_start(wq1[:], wq_d[128:192, :].bitcast(F32R))
                nc.sync.dma_start(x1p[:, c0:c1], x_d[128:192, c0:c1].bitcast(F32R))
            x_pre[0] = (x0p, x1p)
            w9t = wp.tile([128, 27], F32, tag="w9t")
            nc.sync.dma_start(w9t[:], w9_d[:])
            w9 = [w9t[:, 9 * c:9 * (c + 1)] for c in range(3)]
            wptt = wp.tile([HC, 4 * DIM], BF16, tag="wptt")
            nc.sync.dma_start(wptt[:], wp_d[:])
            wpt = [wptt[:, DIM * h:DIM * (h + 1)] for h in range(HEADS)]
            tvt = wp.tile([96, 2], F32, tag="tvt")
            nc.sync.dma_start(tvt[:], tp_d[:])
            tempv = {"a": tvt[:, 0:1], "b": tvt[:, 1:2]}
            amask = wp.tile([96, 96], F32, tag="amask")
            nc.sync.dma_start(amask[:], am_d[:])
            ident = wp.tile([128, 128], F32, tag="ident")
            make_identity(nc, ident[:])
            dgt = {}
            dr8t = {}
            dg8t = {}
            if n_pe:
                dgall = wp.tile([128, 18 * 128], BF16, tag="dgall")
                nc.sync.dma_start(dgall[:], dg_d[:])
                for c in (1, 2):
                    for t in range(9):
                        i = (c - 1) * 9 + t
                        dgt[(c, t)] = dgall[:, 128 * i:128 * (i + 1)]
                f8w = wp.tile([128, 25 * 128], FP8, tag="f8w")
                nc.sync.dma_start(f8w[:], f8_d[:])
                for i in range(14):
                    dr8t[i] = f8w[:, 128 * i:128 * (i + 1)].rearrange("p (k m) -> p k m", k=2)
                for i in range(11):
                    dg8t[i] = f8w[:, 128 * (14 + i):128 * (15 + i)]

            # bf16 v resident in SBUF (rows 64:128 of vb_a carry v chans 0:64)
            vb_a = vsp.tile([128, N], BF16, tag="vb_a", bufs=1)
            vb_b = vsp.tile([128, N], BF16, tag="vb_b", bufs=1)
            # Gram psums share one bank: [96, 384] = cols 0:192 chunk A, 192:384 B
            g_all = pg.tile([96, DIM], F32, tag="g_all")
            g_a = g_all[:, 0:96]
            g_b = g_all[:, 96:192]

            # center tap first: it initializes the accumulator over the full
            # strip; the restricted edge taps then accumulate into subsets.
            TAPS = [(0, 0)] + [(dy, dx) for dy in (-1, 0, 1) for dx in (-1, 0, 1)
                               if (dy, dx) != (0, 0)]

            def _load_x(s):
                us, un = _strip_geom(s)
                ucols = un * W
                x0 = xp.tile([128, 2304], F32R, tag="x0", name=f"x0_{s}")
                x1 = xp.tile([64, 2304], F32R, tag="x1", name=f"x1_{s}")
                cuts = [(ucols * i // 2 // W) * W for i in range(2)] + [ucols]
                for ci in range(2):
                    c0, c1 = cuts[ci], cuts[ci + 1]
                    nc.sync.dma_start(x0[:, c0:c1], x_d[0:128, us * W + c0:us * W + c1].bitcast(F32R))
                    nc.sync.dma_start(x1[:, c0:c1], x_d[128:192, us * W + c0:us * W + c1].bitcast(F32R))
                x_pre[s] = (x0, x1)

            qv_hist = {}
            qt_hist = {}
            _emit_gram_late = [None]
            for s in range(S):
                us, un = _strip_geom(s)
                ucols = un * W
                # ---- prefetch next strip's x; this strip's was loaded earlier ----
                if s + 1 < S:
                    _load_x(s + 1)
                x0, x1 = x_pre[s]
                # ---- qkv matmuls -> u strip (fp32) ----
                # m=0 (pure-q chans) lands as fp8 with a 1-elem leading pad so
                # full-width shifted conv APs never go below offset 0.
                u = [up.tile([128, 2306], FP8, tag="u0", name=f"u_{s}_0")] + \
                    [up.tile([128, 2304], BF16, tag=f"u{m}", name=f"u_{s}_{m}") for m in (1, 2)]
                nc.vector.memset(u[0][:, 0:1], 0.0)
                nc.vector.memset(u[0][:, 1 + ucols:2306], 0.0)
                for m in range(3):
                    doff = 1 if m == 0 else 0
                    for (boff, bn) in _qkv_blocks(ucols):
                        ps = pmm.tile([128, NB], F32, tag="mm")
                        nc.tensor.matmul(ps[:, 0:bn], wq0[:, 128 * m:128 * (m + 1)],
                                         x0[:, boff:boff + bn], start=True, stop=False)
                        nc.tensor.matmul(ps[:, 0:bn], wq1[:, 128 * m:128 * (m + 1)],
                                         x1[:, boff:boff + bn], start=False, stop=True)
                        if m == 2 and (s * 3 + 2) not in pe_atoms:
                            nc.vector.tensor_copy(u[m][:, boff:boff + bn], ps[:, 0:bn])
                        else:
                            nc.scalar.activation(u[m][:, doff + boff:doff + boff + bn], ps[:, 0:bn], AF.Copy)
                # ---- depthwise conv: qv[c, r, w] = sum_t w9[c,t]*u[c, r+dy, w+dx] ----
                qv = [qvp.tile([64, 2 * SCOLS], BF16, tag="qv0", name=f"qv_{s}_0"),
                      qvp.tile([128, SCOLS], BF16, tag="qv1", name=f"qv_{s}_1"),
                      None]
                qv_hist[s] = qv
                for m in range(3):
                    uv = u[m][:, 0:ucols].rearrange("p (h w) -> p h w", h=un, w=W) if m else None
                    ovt = (vb_b[:, s * SCOLS:(s + 1) * SCOLS] if m == 2 else
                           (qv[m][:] if m else None))
                    ov = ovt.rearrange("p (h w) -> p h w", h=SR, w=W) if m else None
                    atom = s * 3 + m
                    if m == 0 and atom in pe_atoms:
                        # fp8 DoubleRow conv, wide layout: DR outputs must sit at
                        # partition base 0, so both 64-chan groups write psum
                        # [64, 1024] (g0 cols 0:512, g1 cols 512:1024).  dy-tap
                        # pairs + (0,-1)/(0,+1) + (center,zero) run full-width;
                        # wraparound cols cancelled by negated-weight matmuls.
                        u0f = u[0][:]
                        APc = type(u0f)
                        ppu = list(list(u0f.ap)[0])

                        def sap(off, dims):
                            return APc(u0f.tensor, off, [ppu] + [list(d) for d in dims])

                        for boff in range(0, SCOLS, NB):
                            r0 = boff // W
                            bh = NB // W
                            ps = pcv.tile([64, 2 * NB], F32, tag="cv", name=f"cps_{s}_0_{boff}")
                            psf = ps[:]
                            ppp = list(list(psf.ap)[0])

                            def spp(off, dims):
                                return APc(psf.tensor, off, [ppp] + [list(d) for d in dims])

                            a0 = SR * s + r0 - us
                            # center (paired with zero k-tile, stride-0 rhs)
                            icen = sap(1 + a0 * W, [[0, 2], [1, NB]])
                            seq = [(ps[:, g * NB:(g + 1) * NB], dr8t[8 + g], icen, DRM)
                                   for g in range(2)]
                            for pi, dx in ((0, -1), (1, 0), (2, 1)):
                                loA = max(r0, max(0, -SR * s + 1))
                                hiA = min(r0 + bh, min(SR, H - SR * s + 1))
                                loB, hiB = r0, min(r0 + bh, min(SR, H - SR * s - 1))
                                lo, hi = max(loA, loB), min(hiA, hiB)
                                ws_o, ws_i, wc = (0, 0, W) if dx == 0 else ((0, 1, W - 1) if dx == 1 else (1, 0, W - 1))
                                if hi > lo:
                                    h = hi - lo
                                    aA = SR * s + lo - 1 - us
                                    i4 = sap(1 + aA * W + dx, [[2 * W, 2], [1, h * W]])
                                    for g in range(2):
                                        seq.append((ps[:, g * NB + (lo - r0) * W:g * NB + (hi - r0) * W],
                                                    dr8t[2 * pi + g], i4, DRM))
                                    if dx == -1:
                                        i3 = sap(aA * W, [[2 * W, 2], [W, h]])
                                        for g in range(2):
                                            seq.append((spp(g * NB + (lo - r0) * W, [[W, h]]),
                                                        dr8t[10 + g], i3, DRM))
                                    elif dx == 1:
                                        i3 = sap(1 + (aA + 1) * W, [[2 * W, 2], [W, h]])
                                        for g in range(2):
                                            seq.append((spp(g * NB + (lo - r0) * W + W - 1, [[W, h]]),
                                                        dr8t[12 + g], i3, DRM))
                                # leftover rows where one dy-tap is clamped
                                for dy_, lo_, hi_ in ((-1, loA, hiA), (1, loB, hiB)):
                                    t_ = (dy_ + 1) * 3 + (dx + 1)
                                    for y0, y1 in ((lo_, lo), (hi, hi_)):
                                        for y in range(y0, y1):
                                            a_ = SR * s + y + dy_ - us
                                            for g in range(2):
                                                seq.append((ps[:, g * NB + (y - r0) * W + ws_o:g * NB + (y - r0) * W + ws_o + wc],
                                                            dg8t[t_][:, g * 64:(g + 1) * 64],
                                                            u[0][:, 1 + a_ * W + ws_i:1 + a_ * W + ws_i + wc], None))
                            # (0,-1)+(0,+1) full-width pair, k-stride 2
                            i4 = sap(a0 * W, [[2, 2], [1, bh * W]])
                            for g in range(2):
                                seq.append((ps[:, g * NB:(g + 1) * NB], dr8t[6 + g], i4, DRM))
                            i3l = sap(a0 * W, [[W, bh]])
                            i3r = sap(1 + (a0 + 1) * W, [[W, bh]])
                            for g in range(2):
                                seq.append((spp(g * NB, [[W, bh]]), dg8t[9][:, g * 64:(g + 1) * 64], i3l, None))
                                seq.append((spp(g * NB + W - 1, [[W, bh]]), dg8t[10][:, g * 64:(g + 1) * 64], i3r, None))
                            for ii, (o_, w_, i_, pm_) in enumerate(seq):
                                nc.tensor.matmul(o_, w_, i_, start=(ii < 2), stop=(ii >= len(seq) - 2),
                                                 perf_mode=pm_, skip_group_check=True)
                            ps3 = ps[:].rearrange("p (g c) -> p g c", g=2)
                            qv03 = qv[0][:].rearrange("p (g c) -> p g c", g=2)
                            nc.scalar.activation(qv03[:, :, boff:boff + NB], ps3, AF.Copy)
                        continue
                    if atom in pe_atoms:
                        # PE: 9 accumulating f32r diag matmuls per 512-block,
                        # shifted/restricted 3D APs give exact zero padding.
                        for boff in range(0, SCOLS, NB):
                            r0 = boff // W
                            ps = pcv.tile([128, NB], F32, tag="cv", name=f"cps_{s}_{m}_{boff}")
                            pv = ps[:].rearrange("p (h w) -> p h w", h=NB // W, w=W)
                            for ti, (dy, dx) in enumerate(TAPS):
                                t = (dy + 1) * 3 + (dx + 1)
                                lo = max(r0, max(0, -SR * s - dy))
                                hi = min(r0 + NB // W, min(SR, H - SR * s - dy))
                                if hi <= lo:
                                    continue
                                hc = hi - lo
                                if dx == 0:
                                    ws_o, ws_i, wc = 0, 0, W
                                elif dx == 1:
                                    ws_o, ws_i, wc = 0, 1, W - 1
                                else:
                                    ws_o, ws_i, wc = 1, 0, W - 1
                                a = SR * s + lo + dy - us
                                o_ap = pv[:, lo - r0:hi - r0, ws_o:ws_o + wc]
                                i_ap = uv[:, a:a + hc, ws_i:ws_i + wc]
                                if dy == 0 and dx == 0:
                                    # flat full-block stream (covers whole psum,
                                    # sets has_written everywhere); center tap
                                    # has lo == r0, so u row a aligns with r0.
                                    o_use = ps[:]
                                    i_use = u[m][:, a * W:a * W + NB]
                                else:
                                    o_use, i_use = o_ap, i_ap
                                nc.tensor.matmul(o_use, dgt[(m, t)], i_use,
                                                 start=(ti == 0), stop=(ti == len(TAPS) - 1))
                            dst_bb = ovt[:, boff:boff + NB] if m == 2 else qv[m][:, boff:boff + NB]
                            nc.scalar.activation(dst_bb, ps[:], AF.Copy)
                        continue
                    stt = nc.gpsimd.scalar_tensor_tensor if atom in gps_atoms else nc.vector.scalar_tensor_tensor
                    # center tap initializes the accumulator on ACT (single-src
                    # scale runs 2x there and frees the DVE)
                    a0 = SR * s - us
                    nc.scalar.activation(ovt, u[m][:, a0 * W:a0 * W + SCOLS], AF.Copy,
                                         scale=w9[m][:, 4:5])
                    for (dy, dx) in TAPS[1:]:
                        t = (dy + 1) * 3 + (dx + 1)
                        r_lo = max(0, -SR * s - dy)
                        r_hi = min(SR, H - SR * s - dy)
                        hc = r_hi - r_lo
                        if dx == 0:
                            ws_o, ws_i, wc = 0, 0, W
                        elif dx == 1:
                            ws_o, ws_i, wc = 0, 1, W - 1
                        else:
                            ws_o, ws_i, wc = 1, 0, W - 1
                        a = SR * s + r_lo + dy - us
                        o_ap = ov[:, r_lo:r_hi, ws_o:ws_o + wc]
                        i_ap = uv[:, a:a + hc, ws_i:ws_i + wc]
                        sc = w9[m][:, t:t + 1]
                        stt(o_ap, i_ap, sc, o_ap, op0=ALU.mult, op1=ALU.add)
                # conv tap order: center (init) must run first; Tile's RAW/WAW
                # tracking keeps the remaining taps ordered on the accumulator.
                # ---- v -> resident bf16 ----
                nc.gpsimd.tensor_copy(vb_a[64:128, s * SCOLS:(s + 1) * SCOLS], qv[1][64:128, :])
                if _emit_gram_late[0] is not None:
                    _issue_gram(nc, qt_hist[_emit_gram_late[0]], g_a, g_b, _emit_gram_late[0])
                    _emit_gram_late[0] = None
                # ---- lagged q^T (xbar DMA transpose); gram emitted at the
                # end of the strip so the PE reaches it a conv-duration later.
                if s >= 1:
                    _issue_transpose(nc, qtp, qv_hist[s - 1], qt_hist, s - 1)
                    _emit_gram_late[0] = s - 1

            # flush the lagged transpose/gram pipeline
            _issue_transpose(nc, qtp, qv_hist[S - 1], qt_hist, S - 1)
            _issue_gram(nc, qt_hist[S - 1], g_a, g_b, S - 1)

            # ================= attention head math =================
            gs_all = wp.tile([96, 2 * 96], F32, tag="gs_all")
            nc.scalar.activation(gs_all[:], g_all[:], AF.Copy)
            gs = {"a": gs_all[:, 0:96], "b": gs_all[:, 96:192]}
            st = {}
            sonly = {}
            for key in ("a", "b"):
                # diag -> norms: d_sq[r] = sum_c G[r, c] * I[r, c]
                dtmp = wp.tile([96, 96], F32, tag=f"dtmp_{key}")
                nc.vector.tensor_mul(dtmp[:], gs[key], ident[0:96, 0:96])
                dsq = wp.tile([96, 1], F32, tag=f"dsq_{key}")
                nc.vector.tensor_reduce(dsq[:], dtmp[:], axis=mybir.AxisListType.X, op=ALU.add)
                dn = wp.tile([96, 1], F32, tag=f"dn_{key}")
                nc.scalar.sqrt(dn[:], dsq[:])
                sv = wp.tile([96, 1], F32, tag=f"sv_{key}")
                nc.vector.reciprocal(sv[:], dn[:])
                sonly[key] = sv
                stv = wp.tile([96, 1], F32, tag=f"stv_{key}")
                nc.vector.tensor_mul(stv[:], sv[:], tempv[key])
                st[key] = stv

            at_bf = {}
            for key in ("a", "b"):
                # mask first: -1e30 on cross-head blocks survives the
                # positive scales/transposes (exp -> 0 either way) and runs
                # concurrently with the norm chain instead of after it.
                gm = wp.tile([96, 96], F32, tag=f"gm_{key}")
                nc.vector.tensor_add(gm[:], gs[key], amask[:])
                # Z = diag(s*temp) @ (G_block + mask)   [96, 96]
                z = wp.tile([96, 96], F32, tag=f"z_{key}")
                nc.scalar.activation(z[:], gm[:], AF.Copy, scale=st[key][:])
                pzt = pcv.tile([128, 512], F32, tag="cv", name="pzt")
                nc.tensor.transpose(pzt[0:96, 0:96], z[:], ident[0:96, 0:96])
                zt = wp.tile([96, 96], F32, tag=f"zt_{key}")
                nc.scalar.activation(zt[:], pzt[0:96, 0:96], AF.Copy, scale=sonly[key][:])
                pl = pcv.tile([128, 512], F32, tag="cv", name="pl")
                nc.tensor.transpose(pl[0:96, 0:96], zt[:], ident[0:96, 0:96])
                e = wp.tile([96, 96], F32, tag=f"e_{key}")
                nc.scalar.activation(e[:], pl[0:96, 0:96], AF.Exp)
                rs = wp.tile([96, 1], F32, tag=f"rs_{key}")
                nc.vector.tensor_reduce(rs[:], e[:], axis=mybir.AxisListType.X, op=ALU.add)
                rc = wp.tile([96, 1], F32, tag=f"rc_{key}")
                nc.vector.reciprocal(rc[:], rs[:])
                ab = wp.tile([96, 96], BF16, tag=f"ab_{key}")
                nc.scalar.activation(ab[:], e[:], AF.Copy, scale=rc[:])
                at_bf[key] = ab

            # move odd-head attn blocks to partition 0 (PE needs 32-aligned base)
            scr1 = wp.tile([HC, HC], BF16, tag="scr1")
            scr3 = wp.tile([HC, HC], BF16, tag="scr3")
            nc.sync.dma_start(scr1[:], at_bf["a"][48:96, 48:96])
            nc.sync.dma_start(scr3[:], at_bf["b"][48:96, 48:96])

            # M_h^T = attn_h^T-contracted with w_projT rows: [48d, 192o]
            mh_s = []
            for h in range(HEADS):
                if h == 0:
                    lhsT = at_bf["a"][0:48, 0:48]
                elif h == 1:
                    lhsT = scr1[:]
                elif h == 2:
                    lhsT = at_bf["b"][0:48, 0:48]
                else:
                    lhsT = scr3[:]
                pm = pcv.tile([128, 512], F32, tag="cv", name=f"pm{h}")
                nc.tensor.matmul(pm[0:HC, 0:DIM], lhsT, wpt[h], start=True, stop=True)
                ms = wp.tile([HC, DIM], BF16, tag=f"mh{h}")
                nc.scalar.activation(ms[:], pm[0:HC, 0:DIM], AF.Copy)
                mh_s.append(ms)

            # stack into final lhsT k-chunks: Ma rows 64:128 = d 0:64 (base
            # partition must match vb_a), Mb = d 64:192
            m_a = wp.tile([128, DIM], BF16, tag="m_a")
            m_b = wp.tile([128, DIM], BF16, tag="m_b")
            nc.sync.dma_start(m_a[64:112, :], mh_s[0][:])
            nc.gpsimd.dma_start(m_a[112:128, :], mh_s[1][0:16, :])
            nc.sync.dma_start(m_b[0:32, :], mh_s[1][16:48, :])
            nc.gpsimd.dma_start(m_b[32:80, :], mh_s[2][:])
            nc.sync.dma_start(m_b[80:128, :], mh_s[3][:])

            # ================= final y = M @ v =================
            YB = 4 * NB  # 2048-col y chunks -> few large output DMAs
            for goff in range(0, N, YB):
                yts = [yp.tile([128, YB], BF16, tag=f"yt{mi}", name=f"yt{mi}_{goff}")
                       for mi in range(2)]
                for sub in range(4):
                    boff = goff + sub * NB
                    for mi, (mp0, mn) in enumerate(((0, 128), (128, 64))):
                        ps = pmm.tile([128, NB], F32, tag="mm")
                        nc.tensor.matmul(ps[0:mn, :], m_a[64:128, mp0:mp0 + mn],
                                         vb_a[64:128, boff:boff + NB], start=True, stop=False)
                        nc.tensor.matmul(ps[0:mn, :], m_b[:, mp0:mp0 + mn],
                                         vb_b[:, boff:boff + NB], start=False, stop=True)
                        dst = yts[mi][0:mn, sub * NB:(sub + 1) * NB]
                        if mi == 0:
                            nc.scalar.activation(dst, ps[0:mn, :], AF.Copy)
                        else:
                            nc.vector.tensor_copy(dst, ps[0:mn, :])
                for mi, (mp0, mn) in enumerate(((0, 128), (128, 64))):
                    hb = YB // 2
                    for hh in range(2):
                        nc.sync.dma_start(y_d[mp0:mp0 + mn, goff + hh * hb:goff + (hh + 1) * hb],
                                          yts[mi][0:mn, hh * hb:(hh + 1) * hb])

    _split_multi_waits(nc)
    return nc


_CACHED = {}


def _get_nc(conv_cfg=(0, 0)):
    key = tuple(conv_cfg)
    if key not in _CACHED:
        _CACHED[key] = build_nc(key)
    return _CACHED[key]


def _host_prep(x, w_qkv, w_dw, w_proj, temperature):
    x = np.ascontiguousarray(np.asarray(x, dtype=np.float32))
    w_qkv = np.asarray(w_qkv, dtype=np.float32)
    w_dw = np.asarray(w_dw, dtype=np.float32)
    w_proj = np.asarray(w_proj, dtype=np.float32)
    temperature = np.asarray(temperature, dtype=np.float32)
    B = x.shape[0]
    xs = x.reshape(B, DIM, N)
    wqkvT = np.ascontiguousarray(w_qkv.T)                      # [192, 384]
    w9 = np.ascontiguousarray(w_dw.reshape(2 * DIM, 9))        # [384, 9]
    w9all = np.zeros((128, 27), np.float32)
    for c in range(3):
        w9all[:, 9 * c:9 * (c + 1)] = w9[128 * c:128 * (c + 1)]
    wprojT = np.ascontiguousarray(w_proj.T.astype(ml_dtypes.bfloat16))  # [192,192]
    wp2 = np.zeros((HC, 4 * DIM), ml_dtypes.bfloat16)
    for h in range(HEADS):
        wp2[:, DIM * h:DIM * (h + 1)] = wprojT[HC * h:HC * (h + 1)]
    tempv = np.repeat(temperature.reshape(HEADS), HC).astype(np.float32)
    tv2 = np.stack([tempv[0:96], tempv[96:192]], axis=1).astype(np.float32)
    return xs, wqkvT, w9, w9all, wp2, tv2


def kernel(x, w_qkv, w_dw, w_proj, temperature, _trace=False, _conv=(16, 0)):
    xs, wqkvT, w9, w9all, wp2, tv2 = _host_prep(x, w_qkv, w_dw, w_proj, temperature)
    B = xs.shape[0]
    nc = _get_nc(_conv)
    in_maps = []
    amask = np.full((96, 96), -1e30, np.float32)
    amask[0:48, 0:48] = 0.0
    amask[48:96, 48:96] = 0.0
    extra = {}
    if _conv[0]:
        dw = np.zeros((18, 128, 128), np.float32)
        for c in (1, 2):
            for t in range(9):
                np.fill_diagonal(dw[(c - 1) * 9 + t], w9[128 * c:128 * (c + 1), t])
        extra["diagw"] = dw.transpose(1, 0, 2).reshape(128, 18 * 128).astype(ml_dtypes.bfloat16)
        # fp8 DoubleRow weights for m=0 (q chans 0:128)
        w0 = w9[0:128].astype(ml_dtypes.float8_e4m3fn).astype(np.float32)
        dg8 = np.zeros((11, 128, 128), np.float32)
        for t in range(9):
            np.fill_diagonal(dg8[t], w0[:, t])
        np.fill_diagonal(dg8[9], -w0[:, 3])
        np.fill_diagonal(dg8[10], -w0[:, 5])
        pairs = [(0, 6), (1, 7), (2, 8), (3, 5), (4, None)]
        dr8 = np.zeros((14, 128, 2, 64), np.float32)
        for pi, (tA, tB) in enumerate(pairs):
            for g in range(2):
                for mc in range(64):
                    p = g * 64 + mc
                    dr8[2 * pi + g, p, 0, mc] = w0[p, tA]
                    if tB is not None:
                        dr8[2 * pi + g, p, 1, mc] = w0[p, tB]
        dr8[10:12] = -dr8[0:2]   # negated dx=-1 pair
        dr8[12:14] = -dr8[4:6]   # negated dx=+1 pair
        f8all = np.concatenate([dr8.reshape(14, 128, 128), dg8], axis=0)
        extra["f8w"] = f8all.transpose(1, 0, 2).reshape(128, 25 * 128).astype(ml_dtypes.float8_e4m3fn)
    for b in range(B):
        m = {"x": xs[b], "wqkvT": wqkvT, "w9": w9all, "wprojT": wp2, "tempv": tv2,
             "amask": amask, **extra}
        in_maps.append(m)
    res = run_bass_kernel_spmd(nc, in_maps, list(range(8)), trace=_trace)
    y = np.stack([np.asarray(res.results[b]["y"]).astype(np.float32) for b in range(B)])
    out = y.reshape(B, DIM, H, W)
    if _trace:
        return out, res
    return out

